# revision 47
# baseline (speedup 1.0000x reference)
"""Causal bag-of-words pooling (running causal mean) on 8 trn2 NeuronCores.

y[b, t, :] = mean(x[b, :t+1, :])  for x of shape (8, 4096, 1024) fp32.

Sharding: data-parallel over B — core i handles batch element i.

Per-core algorithm (T=4096, C=1024, block TB=128), fully streaming:
  For each 128-row block k (2 chunks of 512 channels each):
    MM1 (fp32): psum = UT128.T @ x_k — within-block cumsum via an
        upper-triangular-ones matmul on the TensorEngine.
    MM2 (bf16, K=64, accumulating into the same PSUM group): broadcast-adds
        the running offset acc_k to all 128 rows. acc_k is held as a hi/lo
        bf16 pair (rows 31/63 of a 64-row tile; the sel64 lhsT selects them),
        preserving ~fp32 accuracy at full bf16 PE rate in ONE matmul.
    The next offset acc_{k+1} is psum row 127 of the finished group (cumsum
        through block k). Compute engines can only address base partitions
        0/32/64/96, so the DVE extracts the [96:128] window: hi = bf16 cast
        into acc rows [0:32], lo = residual into rows [32:64].
    A scaled copy on ScalarE applies the per-row 1/(t+1) factor on the way
        out of PSUM (per-partition scale AP).

Scheduling/data-movement notes (these were the big wins):
  - All DMA via gpsimd SWDGE with 4 parallel queues; only full-128-partition
    transfers take its vectorized 16-lane path (descriptors spread across
    all 16 SDMA engines at ~27 GB/s each). Odd partition counts degrade to
    a scalar path pinned on one engine (~16x slower).
  - All input loads are emitted first (Q7 issues in program order, so output
    waits must not gate input issue); 2 MB batches, 4 blocks per tile.
  - PE emission is software-pipelined (PD=3): MM1s of blocks k..k+3 run
    ahead of the chain-blocked MM2 of block k, so the in-order PE never
    stalls on the MM2 -> DVE-extract -> MM2 round trip.
Measured on trn2: ~116 us/core vs ~94 us HBM roofline for the 32 MB/core
of traffic (absmax err ~7e-6 vs fp32 reference).
"""

import sys

import numpy as np

if "/opt/trn_rl_repo" not in sys.path:
    sys.path.insert(0, "/opt/trn_rl_repo")

B, T, C = 8, 4096, 1024
TB = 128                  # rows per block (partition dim)
NB = T // TB              # 32 blocks
FJ = 512                  # matmul moving free dim (PSUM bank = 512 fp32)
NJ = C // FJ              # 2 chunks
XB = 4                    # blocks per input/output DMA (2 MB SWDGE transfers)

_CACHE: dict = {}


def _swq(inst, qnum: int):
    """Route a SWDGE DMA onto qPoolDynamic{qnum} (parallel SWDGE rings)."""
    if qnum:
        inst.ins.queue = f"qPoolDynamic{qnum}"
    return inst


def _consts():
    import ml_dtypes

    # ut128[s, t] = 1 if s <= t : lhsT of the within-block cumsum matmul.
    ut128 = np.triu(np.ones((TB, TB), dtype=np.float32))
    # sel64[k', t] = 1 iff k' in {31, 63}: row 31 of the acc tile is the hi
    # part of the running offset (bf16 of psum row 127), row 63 the lo part;
    # one matmul broadcasts hi+lo to all 128 output rows.
    sel64 = np.zeros((64, TB), dtype=ml_dtypes.bfloat16)
    sel64[31, :] = 1.0
    sel64[63, :] = 1.0
    # recip[p, k] = 1 / (k*TB + p + 1)
    t = (np.arange(NB)[None, :] * TB + np.arange(TB)[:, None] + 1).astype(np.float32)
    recip = (np.float32(1.0) / t).astype(np.float32)
    return ut128, sel64, recip


def _build():
    from concourse import bacc, tile
    import concourse.mybir as mybir

    f32 = mybir.dt.float32
    bf16 = mybir.dt.bfloat16

    nc = bacc.Bacc(
        "TRN2",
        target_bir_lowering=False,
        debug=False,
        enable_asserts=False,
        num_devices=B,
        num_swdge_queues=4,
    )

    x = nc.dram_tensor("x", [T, C], f32, kind="ExternalInput").ap()
    ut128 = nc.dram_tensor("ut128", [TB, TB], f32, kind="ExternalInput").ap()
    sel64 = nc.dram_tensor("sel64", [64, TB], bf16, kind="ExternalInput").ap()
    recip = nc.dram_tensor("recip", [TB, NB], f32, kind="ExternalInput").ap()
    y = nc.dram_tensor("y", [T, C], f32, kind="ExternalOutput").ap()

    with tile.TileContext(nc) as tc:
        with (
            tc.tile_pool(name="consts", bufs=1) as consts,
            tc.tile_pool(name="xin", bufs=4) as xin,
            tc.tile_pool(name="accp", bufs=3) as accp,
            tc.tile_pool(name="outp", bufs=4) as outp,
            tc.tile_pool(name="psC", bufs=8, space="PSUM") as psC,
        ):
            # ut/rec ride the wide SWDGE path (128-partition); sel is only
            # needed at the first MM2 and can trickle in on the idle sync ring
            ut_t = consts.tile([TB, TB], f32, tag="ut")
            nc.gpsimd.dma_start(ut_t[:], ut128[:])
            rec_t = consts.tile([TB, NB], f32, tag="rec")
            nc.gpsimd.dma_start(rec_t[:], recip[:])
            sel_t = consts.tile([64, TB], bf16, tag="sel")
            nc.sync.dma_start(sel_t[:], sel64[:])

            # Issue ALL input DMAs first in gpsimd program order: Q7 issues
            # in-order, so emitting outputs in between would gate input issue
            # on the previous group's full compute chain. Loads are 4 blocks
            # (2 MB) per SWDGE DMA — gpsimd's descriptor generator spreads
            # lines across all 16 SDMA engines, unlike the HWDGE path which
            # serializes DRAM-source reads on one engine.
            xts = []
            for g in range(NB // XB):
                xt = xin.tile([TB, XB * C], f32, tag="x", name=f"x{g}")
                if g == 0:
                    # two 1 MB halves: block 0 lands sooner, so the PE
                    # pipeline starts earlier.
                    h = XB // 2
                    for i in range(2):
                        _swq(
                            nc.gpsimd.dma_start(
                                xt[:, i * h * C:(i + 1) * h * C].rearrange(
                                    "p (f c) -> p f c", f=h
                                ),
                                x[i * h * TB:(i + 1) * h * TB, :].rearrange(
                                    "(f p) c -> p f c", f=h
                                ),
                            ),
                            i,
                        )
                else:
                    _swq(
                        nc.gpsimd.dma_start(
                            xt[:].rearrange("p (f c) -> p f c", f=XB),
                            x[g * XB * TB:(g + 1) * XB * TB, :].rearrange(
                                "(f p) c -> p f c", f=XB
                            ),
                        ),
                        g % 4,
                    )
                xts.append(xt)

            # Software-pipelined emission, depth PD blocks: MM1s of block k are
            # emitted (and run on the in-order PE) ahead of MM2s of block
            # k-PD, so the PE never idles while the offset chain (MM2 ->
            # DVE extract -> next MM2) does its cross-engine round trip.
            # psC bufs=6 holds exactly PD+1 blocks x NJ chunks of open groups.
            PD = 3
            acc = [None] * NJ
            psb = {}
            ots = {}
            for it in range(NB + PD):
                if it < NB:
                    k = it
                    xt = xts[k // XB]
                    xoff = (k % XB) * C
                    for j in range(NJ):
                        ps = psC.tile([TB, FJ], f32, tag="psC")
                        nc.tensor.matmul(
                            ps[:],
                            ut_t[:],
                            xt[:, xoff + j * FJ:xoff + (j + 1) * FJ],
                            start=True,
                            stop=(k == 0),
                        )
                        psb[(k, j)] = ps
                kk = it - PD
                if kk < 0:
                    continue
                g = kk // XB
                if kk % XB == 0:
                    ots[g] = outp.tile([TB, XB * C], f32, tag="out", name="ot")
                ot = ots[g]
                xoff = (kk % XB) * C
                for j in range(NJ):
                    ps = psb.pop((kk, j))
                    if kk > 0:
                        nc.tensor.matmul(
                            ps[:], sel_t[:], acc[j][:],
                            start=False, stop=True,
                        )
                    if kk < NB - 1:
                        # psum row 127 = acc_k + block total = acc_{k+1}.
                        # Compute engines can only start at partitions
                        # 0/32/64/96, so extract the whole [96:128] window
                        # (same DVE cost — free-dim bound). Rows [0:32] of the
                        # acc tile get the bf16 hi part, rows [32:64] the lo
                        # residual; sel_t broadcasts rows 31+63 (= psum row
                        # 127 split hi/lo) in a single accumulating matmul.
                        a2 = accp.tile([64, FJ], bf16, tag=f"acc{j}", name=f"a{j}")
                        nc.vector.tensor_copy(a2[0:32, :], ps[96:128, :])
                        nc.vector.tensor_tensor(
                            a2[32:64, :], ps[96:128, :], a2[0:32, :],
                            mybir.AluOpType.subtract,
                        )
                        acc[j] = a2
                    # scaled copies on ACT only: keeps the DVE exclusively on
                    # the chain's extract ops so they never queue.
                    oc = ot[:, xoff + j * FJ:xoff + (j + 1) * FJ]
                    nc.scalar.mul(oc, ps[:], rec_t[:, kk:kk + 1])
                # One full-128-partition 2MB store per group: SWDGE only takes
                # its 16-lane vectorized path (descriptors spread over all 16
                # SDMA engines) when the partition count is the full 128 — a
                # 127-row store degrades to a scalar path pinned on engine 0.
                if kk % XB == XB - 1:
                    _swq(
                        nc.gpsimd.dma_start(
                            y[g * XB * TB:(g + 1) * XB * TB, :].rearrange(
                                "(f p) c -> p f c", f=XB
                            ),
                            ot[:].rearrange("p (f c) -> p f c", f=XB),
                        ),
                        (g + 1) % 4,
                    )

    nc.compile()

    from concourse.bass_interp import get_hw_module

    nc.m = get_hw_module(nc.m)
    return nc


def _run(x_full: np.ndarray, trace: bool = False):
    from concourse.bass_utils import run_bass_kernel_spmd

    if "nc" not in _CACHE:
        _CACHE["nc"] = _build()
    nc = _CACHE["nc"]

    ut128, sel64, recip = _consts()
    x_full = np.ascontiguousarray(np.asarray(x_full), dtype=np.float32)
    in_maps = [
        {
            "x": np.ascontiguousarray(x_full[i]),
            "ut128": ut128,
            "sel64": sel64,
            "recip": recip,
        }
        for i in range(B)
    ]
    res = run_bass_kernel_spmd(nc, in_maps, core_ids=list(range(B)), trace=trace)
    out = np.stack([np.asarray(res.results[i]["y"]) for i in range(B)], axis=0)
    return out.astype(np.float32), res


def kernel(x: np.ndarray) -> np.ndarray:
    out, _ = _run(x, trace=False)
    return out


# revision 48
# speedup vs baseline: 1.0081x; 1.0081x over previous
"""Causal bag-of-words pooling (running causal mean) on 8 trn2 NeuronCores.

y[b, t, :] = mean(x[b, :t+1, :])  for x of shape (8, 4096, 1024) fp32.

Sharding: data-parallel over B — core i handles batch element i.

Per-core algorithm (T=4096, C=1024, block TB=128), fully streaming:
  For each 128-row block k (2 chunks of 512 channels each):
    MM1 (fp32): psum = UT128.T @ x_k — within-block cumsum via an
        upper-triangular-ones matmul on the TensorEngine.
    MM2 (bf16, K=64, accumulating into the same PSUM group): broadcast-adds
        the running offset acc_k to all 128 rows. acc_k is held as a hi/lo
        bf16 pair (rows 31/63 of a 64-row tile; the sel64 lhsT selects them),
        preserving ~fp32 accuracy at full bf16 PE rate in ONE matmul.
    The next offset acc_{k+1} is psum row 127 of the finished group (cumsum
        through block k). Compute engines can only address base partitions
        0/32/64/96, so the DVE extracts the [96:128] window: hi = bf16 cast
        into acc rows [0:32], lo = residual into rows [32:64].
    A scaled copy on ScalarE applies the per-row 1/(t+1) factor on the way
        out of PSUM (per-partition scale AP).

Scheduling/data-movement notes (these were the big wins):
  - All DMA via gpsimd SWDGE with 4 parallel queues; only full-128-partition
    transfers take its vectorized 16-lane path (descriptors spread across
    all 16 SDMA engines at ~27 GB/s each). Odd partition counts degrade to
    a scalar path pinned on one engine (~16x slower).
  - All input loads are emitted first (Q7 issues in program order, so output
    waits must not gate input issue); 2 MB batches, 4 blocks per tile.
  - PE emission is software-pipelined (PD=3): MM1s of blocks k..k+3 run
    ahead of the chain-blocked MM2 of block k, so the in-order PE never
    stalls on the MM2 -> DVE-extract -> MM2 round trip.
Measured on trn2: ~116 us/core vs ~94 us HBM roofline for the 32 MB/core
of traffic (absmax err ~7e-6 vs fp32 reference).
"""

import sys

import numpy as np

if "/opt/trn_rl_repo" not in sys.path:
    sys.path.insert(0, "/opt/trn_rl_repo")

B, T, C = 8, 4096, 1024
TB = 128                  # rows per block (partition dim)
NB = T // TB              # 32 blocks
FJ = 512                  # matmul moving free dim (PSUM bank = 512 fp32)
NJ = C // FJ              # 2 chunks
XB = 4                    # blocks per input/output DMA (2 MB SWDGE transfers)

_CACHE: dict = {}


def _swq(inst, qnum: int):
    """Route a SWDGE DMA onto qPoolDynamic{qnum} (parallel SWDGE rings)."""
    if qnum:
        inst.ins.queue = f"qPoolDynamic{qnum}"
    return inst


def _consts():
    import ml_dtypes

    # ut128[s, t] = 1 if s <= t : lhsT of the within-block cumsum matmul.
    ut128 = np.triu(np.ones((TB, TB), dtype=np.float32))
    # sel64[k', t] = 1 iff k' in {31, 63}: row 31 of the acc tile is the hi
    # part of the running offset (bf16 of psum row 127), row 63 the lo part;
    # one matmul broadcasts hi+lo to all 128 output rows.
    sel64 = np.zeros((64, TB), dtype=ml_dtypes.bfloat16)
    sel64[31, :] = 1.0
    sel64[63, :] = 1.0
    # recip[p, k] = 1 / (k*TB + p + 1)
    t = (np.arange(NB)[None, :] * TB + np.arange(TB)[:, None] + 1).astype(np.float32)
    recip = (np.float32(1.0) / t).astype(np.float32)
    return ut128, sel64, recip


def _build():
    from concourse import bacc, tile
    import concourse.mybir as mybir

    f32 = mybir.dt.float32
    bf16 = mybir.dt.bfloat16

    nc = bacc.Bacc(
        "TRN2",
        target_bir_lowering=False,
        debug=False,
        enable_asserts=False,
        num_devices=B,
        num_swdge_queues=4,
    )

    x = nc.dram_tensor("x", [T, C], f32, kind="ExternalInput").ap()
    ut128 = nc.dram_tensor("ut128", [TB, TB], f32, kind="ExternalInput").ap()
    sel64 = nc.dram_tensor("sel64", [64, TB], bf16, kind="ExternalInput").ap()
    recip = nc.dram_tensor("recip", [TB, NB], f32, kind="ExternalInput").ap()
    y = nc.dram_tensor("y", [T, C], f32, kind="ExternalOutput").ap()

    with tile.TileContext(nc) as tc:
        with (
            tc.tile_pool(name="consts", bufs=1) as consts,
            tc.tile_pool(name="xin", bufs=4) as xin,
            tc.tile_pool(name="accp", bufs=3) as accp,
            tc.tile_pool(name="outp", bufs=4) as outp,
            tc.tile_pool(name="psC", bufs=8, space="PSUM") as psC,
        ):
            ut_t = consts.tile([TB, TB], f32, tag="ut")
            nc.sync.dma_start(ut_t[:], ut128[:])
            sel_t = consts.tile([64, TB], bf16, tag="sel")
            nc.sync.dma_start(sel_t[:], sel64[:])
            rec_t = consts.tile([TB, NB], f32, tag="rec")
            nc.sync.dma_start(rec_t[:], recip[:])

            # Issue ALL input DMAs first in gpsimd program order: Q7 issues
            # in-order, so emitting outputs in between would gate input issue
            # on the previous group's full compute chain. Loads are 4 blocks
            # (2 MB) per SWDGE DMA — gpsimd's descriptor generator spreads
            # lines across all 16 SDMA engines, unlike the HWDGE path which
            # serializes DRAM-source reads on one engine.
            xts = []
            for g in range(NB // XB):
                xt = xin.tile([TB, XB * C], f32, tag="x", name=f"x{g}")
                if g == 0:
                    # two 1 MB halves: block 0 lands sooner, so the PE
                    # pipeline starts earlier.
                    h = XB // 2
                    for i in range(2):
                        _swq(
                            nc.gpsimd.dma_start(
                                xt[:, i * h * C:(i + 1) * h * C].rearrange(
                                    "p (f c) -> p f c", f=h
                                ),
                                x[i * h * TB:(i + 1) * h * TB, :].rearrange(
                                    "(f p) c -> p f c", f=h
                                ),
                            ),
                            i,
                        )
                else:
                    _swq(
                        nc.gpsimd.dma_start(
                            xt[:].rearrange("p (f c) -> p f c", f=XB),
                            x[g * XB * TB:(g + 1) * XB * TB, :].rearrange(
                                "(f p) c -> p f c", f=XB
                            ),
                        ),
                        g % 4,
                    )
                xts.append(xt)

            # Software-pipelined emission, depth PD blocks: MM1s of block k are
            # emitted (and run on the in-order PE) ahead of MM2s of block
            # k-PD, so the PE never idles while the offset chain (MM2 ->
            # DVE extract -> next MM2) does its cross-engine round trip.
            # psC bufs=6 holds exactly PD+1 blocks x NJ chunks of open groups.
            PD = 3
            acc = [None] * NJ
            psb = {}
            ots = {}
            for it in range(NB + PD):
                if it < NB:
                    k = it
                    xt = xts[k // XB]
                    xoff = (k % XB) * C
                    for j in range(NJ):
                        ps = psC.tile([TB, FJ], f32, tag="psC")
                        nc.tensor.matmul(
                            ps[:],
                            ut_t[:],
                            xt[:, xoff + j * FJ:xoff + (j + 1) * FJ],
                            start=True,
                            stop=(k == 0),
                        )
                        psb[(k, j)] = ps
                kk = it - PD
                if kk < 0:
                    continue
                g = kk // XB
                if kk % XB == 0:
                    ots[g] = outp.tile([TB, XB * C], f32, tag="out", name="ot")
                ot = ots[g]
                xoff = (kk % XB) * C
                for j in range(NJ):
                    ps = psb.pop((kk, j))
                    if kk > 0:
                        nc.tensor.matmul(
                            ps[:], sel_t[:], acc[j][:],
                            start=False, stop=True,
                        )
                    if kk < NB - 1:
                        # psum row 127 = acc_k + block total = acc_{k+1}.
                        # Compute engines can only start at partitions
                        # 0/32/64/96, so extract the whole [96:128] window
                        # (same DVE cost — free-dim bound). Rows [0:32] of the
                        # acc tile get the bf16 hi part, rows [32:64] the lo
                        # residual; sel_t broadcasts rows 31+63 (= psum row
                        # 127 split hi/lo) in a single accumulating matmul.
                        a2 = accp.tile([64, FJ], bf16, tag=f"acc{j}", name=f"a{j}")
                        nc.vector.tensor_copy(a2[0:32, :], ps[96:128, :])
                        nc.vector.tensor_tensor(
                            a2[32:64, :], ps[96:128, :], a2[0:32, :],
                            mybir.AluOpType.subtract,
                        )
                        acc[j] = a2
                    # scaled copies on ACT only: keeps the DVE exclusively on
                    # the chain's extract ops so they never queue.
                    oc = ot[:, xoff + j * FJ:xoff + (j + 1) * FJ]
                    nc.scalar.mul(oc, ps[:], rec_t[:, kk:kk + 1])
                # One full-128-partition 2MB store per group: SWDGE only takes
                # its 16-lane vectorized path (descriptors spread over all 16
                # SDMA engines) when the partition count is the full 128 — a
                # 127-row store degrades to a scalar path pinned on engine 0.
                if kk % XB == XB - 1:
                    _swq(
                        nc.gpsimd.dma_start(
                            y[g * XB * TB:(g + 1) * XB * TB, :].rearrange(
                                "(f p) c -> p f c", f=XB
                            ),
                            ot[:].rearrange("p (f c) -> p f c", f=XB),
                        ),
                        (g + 1) % 4,
                    )

    nc.compile()

    from concourse.bass_interp import get_hw_module

    nc.m = get_hw_module(nc.m)
    return nc


def _run(x_full: np.ndarray, trace: bool = False):
    from concourse.bass_utils import run_bass_kernel_spmd

    if "nc" not in _CACHE:
        _CACHE["nc"] = _build()
    nc = _CACHE["nc"]

    ut128, sel64, recip = _consts()
    x_full = np.ascontiguousarray(np.asarray(x_full), dtype=np.float32)
    in_maps = [
        {
            "x": np.ascontiguousarray(x_full[i]),
            "ut128": ut128,
            "sel64": sel64,
            "recip": recip,
        }
        for i in range(B)
    ]
    res = run_bass_kernel_spmd(nc, in_maps, core_ids=list(range(B)), trace=trace)
    out = np.stack([np.asarray(res.results[i]["y"]) for i in range(B)], axis=0)
    return out.astype(np.float32), res


def kernel(x: np.ndarray) -> np.ndarray:
    out, _ = _run(x, trace=False)
    return out
